# revision 32
# baseline (speedup 1.0000x reference)
"""Block-local self-attention (BLOCK=128, 3-block sliding window + global token 0)
for Trainium2, sharded over 8 NeuronCores by (batch*head).

Full shapes: q/k/v (2, 16, 4096, 64) fp32, mask (2, 1, 1, 4096) fp32 (zeros).
Core c handles 4 consecutive (n*16+h) heads, as 2 "head pairs".

Design notes (ScalarE exp is the bottleneck engine; keep it ~100% busy while
the PE row-tiles the two heads of a pair):
  - Job = (pair, window of 2 query blocks): 32 jobs/core.
  - Scores tile is (128, 2048) fp32 = 4 PSUM banks, SINGLE buffer: head A
    pieces packed in banks 0-1 ([0:768)), head B in banks 2-3 ([1024:1792)).
    The PE co-executes row-tiled matmuls (head A rows 0-63, head B rows
    64-127) out of order, and concurrent writes to the same PSUM bank crash
    the device - the strict A/B bank split makes any overlap safe.
  - exp is split per head (exp_A, then exp_B). Once exp_A(it) drains, banks
    0-1 are free, so job it+1's A-scores run during exp_B(it): the single
    4-bank tile double-buffers at sub-tile granularity and ScalarE never
    idles. ctx pool takes 2 more banks (6/8 used).
  - Global token + softmax denominator are folded into the PE: per head one
    K=2 matmul (lhsT rows = pgq for the 2 query blocks, rhs = per-slot
    [v0, 1] rows) accumulates pgq[q]*[v0,1] into both ctx slots AFTER the
    head's 6 PV matmuls (start=False, stop=True closes the accumulation).
    PSUM col 64 of each slot then holds the full softmax denominator, so
    normalize on DVE is just rt = 1/ctx[..,64]; out = ctx[..,0:64] * rt
    (bf16, fully contiguous 1KB/partition stores every 4 windows).
Query token 0 (attends the full sequence) is host-computed and patched in.
"""

import itertools
import math

import numpy as np
import ml_dtypes

N_, H, T, D = 2, 16, 4096, 64
B = 128
NB = T // B            # 32 key/query blocks
HPC = 4                # heads per core
NCORES = 8
GQ = 2                 # query blocks per job
NWIN = NB // GQ        # 16 windows per pair
SCALE = 1.0 / math.sqrt(D)
BANK = 512             # fp32 elements per PSUM bank (per partition)
VW = D + 1             # vt free width: 64 d + 1 ones
HBOFF = 1024           # head B's column offset in the scores tile (bank 2)


def _window_pieces(w):
    """Pieces for window w: list of (j, qlo, qhi, n), q blocks absolute."""
    qb0, qb1 = GQ * w, GQ * w + GQ - 1
    out = []
    for j in range(max(0, qb0 - 1), min(NB - 1, qb1 + 1) + 1):
        qlo = max(qb0, j - 1)
        qhi = min(qb1, j + 1)
        out.append((j, qlo, qhi, (qhi - qlo + 1) * B))
    return out


def _pack_offsets(sizes):
    """Pack piece sizes contiguously from 0 s.t. no piece crosses a 512-elem
    PSUM bank boundary. Returns (offsets, total)."""
    for perm in itertools.permutations(range(len(sizes))):
        off = 0
        offs = [0] * len(sizes)
        ok = True
        for i in perm:
            sz = sizes[i]
            if off // BANK != (off + sz - 1) // BANK:
                ok = False
                break
            offs[i] = off
            off += sz
        if ok:
            return offs, off
    raise ValueError(f"cannot pack {sizes}")


_NC_CACHE = {}


def _build_nc():
    if "nc" in _NC_CACHE:
        return _NC_CACHE["nc"]

    import concourse.bacc as bacc
    import concourse.mybir as mybir
    import concourse.tile as tile

    dt = mybir.dt
    F32, BF16 = dt.float32, dt.bfloat16
    SCW = 4 * BANK  # scores tile columns (4 banks)

    nc = bacc.Bacc("TRN2", target_bir_lowering=False, debug=False)
    NS = 2 * GQ  # ctx slots per job: (headA c0, headA c1, headB c0, headB c1)
    qt_d = nc.dram_tensor("qt", [2, 128, T], BF16, kind="ExternalInput")
    kt_d = nc.dram_tensor("kt", [2, 128, T], BF16, kind="ExternalInput")
    vt_d = nc.dram_tensor("vt", [HPC, 128, NB, VW], BF16, kind="ExternalInput")
    pgq_d = nc.dram_tensor("pgq", [2, NS, NWIN, B], BF16, kind="ExternalInput")
    v0one_d = nc.dram_tensor("v0one", [2, NS, NS * VW], BF16, kind="ExternalInput")
    o_d = nc.dram_tensor("o", [HPC, 128, NB, D], BF16, kind="ExternalOutput")

    with tile.TileContext(nc) as tc:
        with (
            tc.tile_pool(name="singles", bufs=1) as singles,
            tc.tile_pool(name="pp", bufs=2) as pp,
            tc.tile_pool(name="rtp", bufs=2) as rtp,
            tc.tile_pool(name="outp", bufs=2) as outp,
            tc.tile_pool(name="spsum", bufs=1, space="PSUM") as spsum,
            tc.tile_pool(name="cpsum", bufs=2, space="PSUM") as cpsum,
        ):
            # Warm the ScalarE exp table first, during the DMA ramp.
            warm_in = singles.tile([1, 8], F32, tag="warm_in")
            nc.vector.memset(warm_in[:, :], 0.0)
            warm_out = singles.tile([1, 8], BF16, tag="warm_out")
            nc.scalar.activation(
                out=warm_out[:, :],
                in_=warm_in[:, :],
                func=mybir.ActivationFunctionType.Exp,
            )

            # Inputs: critical-first (pair 0's kt/qt + the tiny globals) on
            # the idle SP queue, bulk on gpsimd SWDGE.
            vt = [None] * HPC
            SPL = 8 * B  # first chunk: kt/qt blocks 0-7 (covers 4 windows)
            kt0 = singles.tile([128, T], BF16, tag="kt0")
            qt0 = singles.tile([128, T], BF16, tag="qt0")
            kt1 = singles.tile([128, T], BF16, tag="kt1")
            qt1 = singles.tile([128, T], BF16, tag="qt1")
            kt_pair, qt_pair = [kt0, kt1], [qt0, qt1]
            pgq = [
                singles.tile([NS, NWIN, B], BF16, tag=f"pgq{p}", name=f"pgq{p}")
                for p in range(2)
            ]
            v0one = [
                singles.tile([NS, NS * VW], BF16, tag=f"v0one{p}", name=f"v0one{p}")
                for p in range(2)
            ]

            nc.sync.dma_start(out=kt0[:, 0:SPL], in_=kt_d.ap()[0, :, 0:SPL])
            nc.sync.dma_start(out=qt0[:, 0:SPL], in_=qt_d.ap()[0, :, 0:SPL])
            for p in range(2):
                nc.sync.dma_start(out=pgq[p][:, :, :], in_=pgq_d.ap()[p])
                nc.sync.dma_start(out=v0one[p][:, :], in_=v0one_d.ap()[p])
            for h in (0, 1):
                vt_h = singles.tile([128, NB, VW], BF16, tag=f"vt{h}", name=f"vt{h}")
                nc.gpsimd.dma_start(out=vt_h[:, :, :], in_=vt_d.ap()[h])
                vt[h] = vt_h
            nc.gpsimd.dma_start(out=kt0[:, SPL:T], in_=kt_d.ap()[0, :, SPL:T])
            nc.gpsimd.dma_start(out=qt0[:, SPL:T], in_=qt_d.ap()[0, :, SPL:T])
            for h in (2, 3):
                vt_h = singles.tile([128, NB, VW], BF16, tag=f"vt{h}", name=f"vt{h}")
                nc.gpsimd.dma_start(out=vt_h[:, :, :], in_=vt_d.ap()[h])
                vt[h] = vt_h
            nc.gpsimd.dma_start(out=kt1[:, 0:T], in_=kt_d.ap()[1, :, 0:T])
            nc.gpsimd.dma_start(out=qt1[:, 0:T], in_=qt_d.ap()[1, :, 0:T])

            packs = []
            for w in range(NWIN):
                pieces = _window_pieces(w)
                offs, tot = _pack_offsets([p[3] for p in pieces])
                packs.append((pieces, offs, tot))

            jobs = [(pair, w) for pair in range(2) for w in range(NWIN)]
            state = {}
            stage_cur = {}
            for it in range(len(jobs) + 2):
                if it < len(jobs):
                    pair, w = jobs[it]
                    qt, kt = qt_pair[pair], kt_pair[pair]
                    pieces, offs, tot = packs[w]
                    sc = spsum.tile([128, SCW], F32, tag="sc")
                    P = pp.tile([128, SCW], BF16, tag="p")
                    # Head A pieces (PE rows 0-63, PSUM banks 0-1), its exp,
                    # then head B (rows 64-127, banks 2-3) and its exp. exp_A
                    # frees banks 0-1 so the NEXT job's A block runs during
                    # exp_B; A/B co-execution on the PE is bank-disjoint.
                    for hb, dlo in ((0, 0), (HBOFF, 64)):
                        for (j, qlo, qhi, n), off in zip(pieces, offs):
                            nc.tensor.matmul(
                                out=sc[:, hb + off : hb + off + n],
                                lhsT=kt[dlo : dlo + 64, j * B : (j + 1) * B],
                                rhs=qt[dlo : dlo + 64, qlo * B : (qhi + 1) * B],
                                start=True,
                                stop=True,
                            )
                        nc.scalar.activation(
                            out=P[:, hb : hb + tot],
                            in_=sc[:, hb : hb + tot],
                            func=mybir.ActivationFunctionType.Exp,
                            scale=SCALE,
                        )
                    state[it] = (pair, w, P)
                if 0 <= it - 1 < len(jobs):
                    pair, w, P = state[it - 1]
                    pieces, offs, tot = packs[w]
                    off_of = {j: (off, qlo) for (j, qlo, qhi, n), off in zip(pieces, offs)}
                    ctx = cpsum.tile([128, 2 * GQ, VW], F32, tag="ctx")
                    # One accumulation group for the whole ctx bank: the K=NS
                    # global matmul opens it (start=True over all slots),
                    # writing pgq[q]*[v0,1] into each slot (global-token term
                    # + denominator seed); the 24 PV matmuls then accumulate
                    # back-to-back and the last one closes the group.
                    nc.tensor.matmul(
                        out=ctx[:, :, :],
                        lhsT=pgq[pair][:, w, :],
                        rhs=v0one[pair][:, :],
                        start=True,
                        stop=False,
                    )
                    pv = []
                    for hi in (0, 1):
                        h = 2 * pair + hi
                        hb = hi * HBOFF
                        for c in range(GQ):
                            cb = GQ * w + c  # absolute q block
                            js = [j for (j, qlo, qhi, n) in pieces if qlo <= cb <= qhi]
                            for j in js:
                                off, qlo = off_of[j]
                                col = hb + off + (cb - qlo) * B
                                pv.append((GQ * hi + c, col, h, j))
                    for i, (slot, col, h, j) in enumerate(pv):
                        nc.tensor.matmul(
                            out=ctx[:, slot, :],
                            lhsT=P[:, col : col + B],
                            rhs=vt[h][:, j, :],
                            start=False,
                            stop=(i == len(pv) - 1),
                        )
                    state[it - 1] = (pair, w, ctx)
                if 0 <= it - 2 < len(jobs):
                    pair, w, ctx = state.pop(it - 2)
                    wi = w % 4
                    rt = rtp.tile([128, 2 * GQ], F32, tag="rt")
                    nc.vector.reciprocal(out=rt[:, :], in_=ctx[:, :, D])
                    for hi in (0, 1):
                        h = 2 * pair + hi
                        if wi == 0:
                            stage_cur[h] = outp.tile(
                                [128, 4 * GQ, D], BF16, tag=f"st{h}", name=f"st{h}"
                            )
                        stage = stage_cur[h]
                        nc.vector.tensor_mul(
                            out=stage[:, GQ * wi : GQ * wi + GQ, :],
                            in0=ctx[:, GQ * hi : GQ * hi + GQ, 0:D],
                            in1=rt[:, GQ * hi : GQ * hi + GQ].broadcast_to(
                                [128, GQ, D]
                            ),
                        )
                        if wi == 3:
                            b0 = (w - 3) * GQ
                            nc.sync.dma_start(
                                out=o_d.ap()[h, :, b0 : b0 + 4 * GQ],
                                in_=stage[:, :, :],
                            )

    nc.compile()
    _NC_CACHE["nc"] = nc
    return nc


def _host_globals(query, key, value):
    """Host-side tiny pieces: pg = exp(scale * K0 . Q) (zeroed for the first
    two query blocks), and o0 = full-sequence attention output for query 0
    (token 0 masked out, as the reference does via attention_mask[..., 0])."""
    q = np.asarray(query, np.float32)
    k = np.asarray(key, np.float32)
    v = np.asarray(value, np.float32)
    k0 = k[:, :, 0, :]  # (n, h, d)
    sg = np.einsum("nhd,nhtd->nht", k0, q) * SCALE
    pg = np.exp(sg)
    pg[:, :, : 2 * B] = 0.0

    q0 = q[:, :, 0, :]  # (n, h, d)
    s0 = np.einsum("nhd,nhtd->nht", q0, k) * SCALE
    s0[:, :, 0] = -np.inf
    s0 -= s0.max(axis=-1, keepdims=True)
    p0 = np.exp(s0)
    p0 /= p0.sum(axis=-1, keepdims=True)
    o0 = np.einsum("nht,nhtd->nhd", p0, v)
    return pg, o0


def kernel(query_layer, key_layer, value_layer, attention_mask):
    from concourse.bass_utils import run_bass_kernel_spmd

    n, h, t, d = query_layer.shape
    assert (n, h, t, d) == (N_, H, T, D)

    q = np.asarray(query_layer, np.float32)
    k = np.asarray(key_layer, np.float32)
    v = np.asarray(value_layer, np.float32)
    pg, o0 = _host_globals(q, k, v)

    bf16 = ml_dtypes.bfloat16
    qf = q.reshape(n * h, T, D)
    kf = k.reshape(n * h, T, D)
    vf = v.reshape(n * h, T, D)

    # qt/kt: per pair of heads, (128, T) bf16 = [headA dT; headB dT]
    qt_all = np.ascontiguousarray(
        qf.astype(bf16).transpose(0, 2, 1).reshape(n * h // 2, 128, T)
    )
    kt_all = np.ascontiguousarray(
        kf.astype(bf16).transpose(0, 2, 1).reshape(n * h // 2, 128, T)
    )
    # vt: (head, 128, NB, 65): [..., 0:64]=V, [..., 64]=ones
    vt_all = np.empty((n * h, 128, NB, VW), bf16)
    vt_all[:, :, :, 0:D] = vf.reshape(n * h, NB, B, D).transpose(0, 2, 1, 3)
    vt_all[:, :, :, D] = np.ones((), bf16)
    # pgq: (pair, 4, NWIN, 128): stationary rows of the K=4 global matmul:
    # row 2*hi+c of window w = pg[head hi of pair, query block GQ*w+c]
    NS = 2 * GQ
    pgq_all = (
        pg.reshape(n * h // 2, 2, NWIN, GQ, B)
        .transpose(0, 1, 3, 2, 4)
        .reshape(n * h // 2, NS, NWIN, B)
        .astype(bf16)
    )
    pgq_all = np.ascontiguousarray(pgq_all)
    # v0one: (pair, 4, 4*65): row r = [v0_head(r), 1] at slot r's cols, else 0
    v0one_all = np.zeros((n * h // 2, NS, NS * VW), bf16)
    v0f = vf[:, 0, :].reshape(n * h // 2, 2, D).astype(bf16)
    for r in range(NS):
        hi = r // GQ
        v0one_all[:, r, r * VW : r * VW + D] = v0f[:, hi, :]
        v0one_all[:, r, r * VW + D] = np.ones((), bf16)

    in_maps = []
    for c in range(NCORES):
        s = slice(HPC * c, HPC * (c + 1))
        sp = slice(HPC // 2 * c, HPC // 2 * (c + 1))
        in_maps.append(
            {
                "qt": np.ascontiguousarray(qt_all[sp]),
                "kt": np.ascontiguousarray(kt_all[sp]),
                "vt": np.ascontiguousarray(vt_all[s]),
                "pgq": np.ascontiguousarray(pgq_all[sp]),
                "v0one": np.ascontiguousarray(v0one_all[sp]),
            }
        )

    nc = _build_nc()
    res = run_bass_kernel_spmd(nc, in_maps, core_ids=list(range(NCORES)))
    _NC_CACHE["last_result"] = res
    out = np.concatenate([r["o"] for r in res.results], axis=0)
    out = out.astype(np.float32)
    out = out.reshape(n * h, 128, NB, D).transpose(0, 2, 1, 3).reshape(n, h, T, D)
    out = np.ascontiguousarray(out)
    out[:, :, 0, :] = o0
    return out


# revision 33
# speedup vs baseline: 1.0751x; 1.0751x over previous
"""Block-local self-attention (BLOCK=128, 3-block sliding window + global token 0)
for Trainium2, sharded over 8 NeuronCores by (batch*head).

Full shapes: q/k/v (2, 16, 4096, 64) fp32, mask (2, 1, 1, 4096) fp32 (zeros).
Core c handles 4 consecutive (n*16+h) heads, as 2 "head pairs".

Design notes:
  - Job = (pair, window of 4 query blocks): 16 jobs/core. Scores tile is
    (128, 3072) fp32 = 6 PSUM banks, single buffer: head A pieces packed in
    banks 0-2 ([0:1536)), head B in banks 3-5. The PE co-executes matmuls
    with disjoint row ranges out of order, and concurrent writes to the same
    PSUM bank crash the device - the strict A/B bank split keeps any
    (head A rows 0-63) x (head B rows 64-127) overlap safe.
  - exp is split per head (exp_A, then exp_B on ScalarE). Once exp_A(it)
    drains, banks 0-2 are free, so job it+1's A-scores run during exp_B(it):
    the single 6-bank tile double-buffers at sub-tile granularity and the
    ScalarE exp stream never waits on scores (the baseline serialized them).
  - Long pieces (up to 384 cols) keep the PE streaming near full rate;
    instruction count is minimized (22-24 score matmuls + 24 PV + 2 globals
    per job pair).
  - Global token + softmax denominator are folded into the PE: per head one
    K=4 matmul (stationary rows = pgq for the 4 query blocks, rhs row r =
    [v0, 1] at slot r's columns) OPENS the ctx accumulation group writing
    pgq[q]*[v0,1] into each slot; the head's 12 PV matmuls (rhs = vt with a
    trailing ones column) accumulate after it and the last closes the group.
    PSUM col 64 of each slot then holds the softmax denominator, so
    normalize on DVE is just rt = 1/ctx[..,64]; out = ctx[..,0:64] * rt
    (bf16, fully contiguous 1KB/partition stores every 2 windows).
Query token 0 (attends the full sequence) is host-computed and patched in.
"""

import itertools
import math

import numpy as np
import ml_dtypes

N_, H, T, D = 2, 16, 4096, 64
B = 128
NB = T // B            # 32 key/query blocks
HPC = 4                # heads per core
NCORES = 8
GQ = 4                 # query blocks per job
NWIN = NB // GQ        # 8 windows per pair
SCALE = 1.0 / math.sqrt(D)
BANK = 512             # fp32 elements per PSUM bank (per partition)
VW = D + 1             # vt free width: 64 d + 1 ones
HBOFF = 3 * BANK       # head B's column offset in the scores tile (bank 3)


def _window_pieces(w):
    """Pieces for window w: list of (j, qlo, qhi, n), q blocks absolute."""
    qb0, qb1 = GQ * w, GQ * w + GQ - 1
    out = []
    for j in range(max(0, qb0 - 1), min(NB - 1, qb1 + 1) + 1):
        qlo = max(qb0, j - 1)
        qhi = min(qb1, j + 1)
        out.append((j, qlo, qhi, (qhi - qlo + 1) * B))
    return out


def _pack_offsets(sizes):
    """Pack piece sizes contiguously from 0 s.t. no piece crosses a 512-elem
    PSUM bank boundary. Returns (offsets, total)."""
    for perm in itertools.permutations(range(len(sizes))):
        off = 0
        offs = [0] * len(sizes)
        ok = True
        for i in perm:
            sz = sizes[i]
            if off // BANK != (off + sz - 1) // BANK:
                ok = False
                break
            offs[i] = off
            off += sz
        if ok:
            return offs, off
    raise ValueError(f"cannot pack {sizes}")


_NC_CACHE = {}


def _build_nc():
    if "nc" in _NC_CACHE:
        return _NC_CACHE["nc"]

    import concourse.bacc as bacc
    import concourse.mybir as mybir
    import concourse.tile as tile

    dt = mybir.dt
    F32, BF16 = dt.float32, dt.bfloat16
    SCW = 6 * BANK  # scores tile columns (6 banks)

    nc = bacc.Bacc("TRN2", target_bir_lowering=False, debug=False)
    qt_d = nc.dram_tensor("qt", [2, 128, T], BF16, kind="ExternalInput")
    kt_d = nc.dram_tensor("kt", [2, 128, T], BF16, kind="ExternalInput")
    vt_d = nc.dram_tensor("vt", [HPC, 128, NB, VW], BF16, kind="ExternalInput")
    pgq_d = nc.dram_tensor("pgq", [HPC, GQ, NWIN, B], BF16, kind="ExternalInput")
    v0one_d = nc.dram_tensor("v0one", [HPC, GQ, GQ * VW], BF16, kind="ExternalInput")
    o_d = nc.dram_tensor("o", [HPC, 128, NB, D], BF16, kind="ExternalOutput")

    with tile.TileContext(nc) as tc:
        with (
            tc.tile_pool(name="singles", bufs=1) as singles,
            tc.tile_pool(name="pp", bufs=2) as pp,
            tc.tile_pool(name="rtp", bufs=2) as rtp,
            tc.tile_pool(name="outp", bufs=2) as outp,
            tc.tile_pool(name="spsum", bufs=1, space="PSUM") as spsum,
            tc.tile_pool(name="cpsum", bufs=1, space="PSUM") as cpsum,
        ):
            # Warm the ScalarE exp table first, during the DMA ramp.
            warm_in = singles.tile([1, 8], F32, tag="warm_in")
            nc.vector.memset(warm_in[:, :], 0.0)
            warm_out = singles.tile([1, 8], BF16, tag="warm_out")
            nc.scalar.activation(
                out=warm_out[:, :],
                in_=warm_in[:, :],
                func=mybir.ActivationFunctionType.Exp,
            )

            # Inputs: critical-first (pair 0's kt/qt + the tiny globals) on
            # the idle SP queue, bulk on gpsimd SWDGE.
            vt = [None] * HPC
            SPL = 8 * B  # first chunk: kt/qt blocks 0-7 (covers 2 windows)
            kt0 = singles.tile([128, T], BF16, tag="kt0")
            qt0 = singles.tile([128, T], BF16, tag="qt0")
            kt1 = singles.tile([128, T], BF16, tag="kt1")
            qt1 = singles.tile([128, T], BF16, tag="qt1")
            kt_pair, qt_pair = [kt0, kt1], [qt0, qt1]
            pgq = [
                singles.tile([GQ, NWIN, B], BF16, tag=f"pgq{h}", name=f"pgq{h}")
                for h in range(HPC)
            ]
            v0one = [
                singles.tile([GQ, GQ * VW], BF16, tag=f"v0one{h}", name=f"v0one{h}")
                for h in range(HPC)
            ]

            nc.sync.dma_start(out=kt0[:, 0:SPL], in_=kt_d.ap()[0, :, 0:SPL])
            nc.sync.dma_start(out=qt0[:, 0:SPL], in_=qt_d.ap()[0, :, 0:SPL])
            for h in range(HPC):
                nc.sync.dma_start(out=pgq[h][:, :, :], in_=pgq_d.ap()[h])
                nc.sync.dma_start(out=v0one[h][:, :], in_=v0one_d.ap()[h])
            for h in (0, 1):
                vt_h = singles.tile([128, NB, VW], BF16, tag=f"vt{h}", name=f"vt{h}")
                nc.gpsimd.dma_start(out=vt_h[:, :, :], in_=vt_d.ap()[h])
                vt[h] = vt_h
            nc.gpsimd.dma_start(out=kt0[:, SPL:T], in_=kt_d.ap()[0, :, SPL:T])
            nc.gpsimd.dma_start(out=qt0[:, SPL:T], in_=qt_d.ap()[0, :, SPL:T])
            for h in (2, 3):
                vt_h = singles.tile([128, NB, VW], BF16, tag=f"vt{h}", name=f"vt{h}")
                nc.gpsimd.dma_start(out=vt_h[:, :, :], in_=vt_d.ap()[h])
                vt[h] = vt_h
            nc.gpsimd.dma_start(out=kt1[:, 0:T], in_=kt_d.ap()[1, :, 0:T])
            nc.gpsimd.dma_start(out=qt1[:, 0:T], in_=qt_d.ap()[1, :, 0:T])

            packs = []
            for w in range(NWIN):
                pieces = _window_pieces(w)
                offs, tot = _pack_offsets([p[3] for p in pieces])
                packs.append((pieces, offs, tot))

            jobs = [(pair, w) for pair in range(2) for w in range(NWIN)]
            state = {}
            stage_cur = {}
            for it in range(len(jobs) + 2):
                if it < len(jobs):
                    pair, w = jobs[it]
                    qt, kt = qt_pair[pair], kt_pair[pair]
                    pieces, offs, tot = packs[w]
                    sc = spsum.tile([128, SCW], F32, tag="sc")
                    P = pp.tile([128, SCW], BF16, tag="p")
                    # Head A block (PE rows 0-63, banks 0-2) and its exp,
                    # then head B (rows 64-127, banks 3-5) and its exp. exp_A
                    # frees banks 0-2 so the NEXT job's A block runs during
                    # exp_B; any A/B co-execution on the PE is bank-disjoint.
                    for hb, dlo in ((0, 0), (HBOFF, 64)):
                        for (j, qlo, qhi, n), off in zip(pieces, offs):
                            nc.tensor.matmul(
                                out=sc[:, hb + off : hb + off + n],
                                lhsT=kt[dlo : dlo + 64, j * B : (j + 1) * B],
                                rhs=qt[dlo : dlo + 64, qlo * B : (qhi + 1) * B],
                                start=True,
                                stop=True,
                            )
                        nc.scalar.activation(
                            out=P[:, hb : hb + tot],
                            in_=sc[:, hb : hb + tot],
                            func=mybir.ActivationFunctionType.Exp,
                            scale=SCALE,
                        )
                    state[it] = (pair, w, P)
                if 0 <= it - 1 < len(jobs):
                    pair, w, P = state[it - 1]
                    pieces, offs, tot = packs[w]
                    off_of = {j: (off, qlo) for (j, qlo, qhi, n), off in zip(pieces, offs)}
                    ctxs = []
                    for hi in (0, 1):
                        h = 2 * pair + hi
                        hb = hi * HBOFF
                        ctx = cpsum.tile(
                            [128, GQ, VW], F32, tag=f"ctx{hi}", name=f"ctx{hi}"
                        )
                        # K=4 global matmul opens the bank's accumulation
                        # group: slot c += pgq[block c][q] * [v0, 1].
                        nc.tensor.matmul(
                            out=ctx[:, :, :],
                            lhsT=pgq[h][:, w, :],
                            rhs=v0one[h][:, :],
                            start=True,
                            stop=False,
                        )
                        pv = []
                        for c in range(GQ):
                            cb = GQ * w + c  # absolute q block
                            js = [j for (j, qlo, qhi, n) in pieces if qlo <= cb <= qhi]
                            for j in js:
                                off, qlo = off_of[j]
                                pv.append((c, hb + off + (cb - qlo) * B, j))
                        for i, (c, col, j) in enumerate(pv):
                            nc.tensor.matmul(
                                out=ctx[:, c, :],
                                lhsT=P[:, col : col + B],
                                rhs=vt[h][:, j, :],
                                start=False,
                                stop=(i == len(pv) - 1),
                            )
                        ctxs.append(ctx)
                    state[it - 1] = (pair, w, ctxs)
                if 0 <= it - 2 < len(jobs):
                    pair, w, ctxs = state.pop(it - 2)
                    wi = w % 2
                    for hi in (0, 1):
                        h = 2 * pair + hi
                        ctx = ctxs[hi]
                        if wi == 0:
                            stage_cur[h] = outp.tile(
                                [128, 2 * GQ, D], BF16, tag=f"st{h}", name=f"st{h}"
                            )
                        stage = stage_cur[h]
                        rt = rtp.tile([128, GQ], F32, tag="rt")
                        nc.vector.reciprocal(out=rt[:, :], in_=ctx[:, :, D])
                        nc.vector.tensor_mul(
                            out=stage[:, GQ * wi : GQ * wi + GQ, :],
                            in0=ctx[:, :, 0:D],
                            in1=rt[:, :].broadcast_to([128, GQ, D]),
                        )
                        if wi == 1:
                            b0 = (w - 1) * GQ
                            nc.sync.dma_start(
                                out=o_d.ap()[h, :, b0 : b0 + 2 * GQ],
                                in_=stage[:, :, :],
                            )

    nc.compile()
    _NC_CACHE["nc"] = nc
    return nc


def _host_globals(query, key, value):
    """Host-side tiny pieces: pg = exp(scale * K0 . Q) (zeroed for the first
    two query blocks), and o0 = full-sequence attention output for query 0
    (token 0 masked out, as the reference does via attention_mask[..., 0])."""
    q = np.asarray(query, np.float32)
    k = np.asarray(key, np.float32)
    v = np.asarray(value, np.float32)
    k0 = k[:, :, 0, :]  # (n, h, d)
    sg = np.einsum("nhd,nhtd->nht", k0, q) * SCALE
    pg = np.exp(sg)
    pg[:, :, : 2 * B] = 0.0

    q0 = q[:, :, 0, :]  # (n, h, d)
    s0 = np.einsum("nhd,nhtd->nht", q0, k) * SCALE
    s0[:, :, 0] = -np.inf
    s0 -= s0.max(axis=-1, keepdims=True)
    p0 = np.exp(s0)
    p0 /= p0.sum(axis=-1, keepdims=True)
    o0 = np.einsum("nht,nhtd->nhd", p0, v)
    return pg, o0


def kernel(query_layer, key_layer, value_layer, attention_mask):
    from concourse.bass_utils import run_bass_kernel_spmd

    n, h, t, d = query_layer.shape
    assert (n, h, t, d) == (N_, H, T, D)

    q = np.asarray(query_layer, np.float32)
    k = np.asarray(key_layer, np.float32)
    v = np.asarray(value_layer, np.float32)
    pg, o0 = _host_globals(q, k, v)

    bf16 = ml_dtypes.bfloat16
    qf = q.reshape(n * h, T, D)
    kf = k.reshape(n * h, T, D)
    vf = v.reshape(n * h, T, D)

    # qt/kt: per pair of heads, (128, T) bf16 = [headA dT; headB dT]
    qt_all = np.ascontiguousarray(
        qf.astype(bf16).transpose(0, 2, 1).reshape(n * h // 2, 128, T)
    )
    kt_all = np.ascontiguousarray(
        kf.astype(bf16).transpose(0, 2, 1).reshape(n * h // 2, 128, T)
    )
    # vt: (head, 128, NB, 65): [..., 0:64]=V, [..., 64]=ones
    vt_all = np.empty((n * h, 128, NB, VW), bf16)
    vt_all[:, :, :, 0:D] = vf.reshape(n * h, NB, B, D).transpose(0, 2, 1, 3)
    vt_all[:, :, :, D] = np.ones((), bf16)
    # pgq: (head, GQ, NWIN, 128): stationary rows of the K=4 global matmul:
    # row c of window w = pg[head, query block GQ*w + c]
    pgq_all = np.ascontiguousarray(
        pg.reshape(n * h, NWIN, GQ, B).transpose(0, 2, 1, 3).astype(bf16)
    )
    # v0one: (head, GQ, GQ*65): row c = [v0, 1] at slot c's cols, else 0
    v0one_all = np.zeros((n * h, GQ, GQ * VW), bf16)
    v0b = vf[:, 0, :].astype(bf16)
    for c in range(GQ):
        v0one_all[:, c, c * VW : c * VW + D] = v0b
        v0one_all[:, c, c * VW + D] = np.ones((), bf16)

    in_maps = []
    for c in range(NCORES):
        s = slice(HPC * c, HPC * (c + 1))
        sp = slice(HPC // 2 * c, HPC // 2 * (c + 1))
        in_maps.append(
            {
                "qt": np.ascontiguousarray(qt_all[sp]),
                "kt": np.ascontiguousarray(kt_all[sp]),
                "vt": np.ascontiguousarray(vt_all[s]),
                "pgq": np.ascontiguousarray(pgq_all[s]),
                "v0one": np.ascontiguousarray(v0one_all[s]),
            }
        )

    nc = _build_nc()
    res = run_bass_kernel_spmd(nc, in_maps, core_ids=list(range(NCORES)))
    _NC_CACHE["last_result"] = res
    out = np.concatenate([r["o"] for r in res.results], axis=0)
    out = out.astype(np.float32)
    out = out.reshape(n * h, 128, NB, D).transpose(0, 2, 1, 3).reshape(n, h, T, D)
    out = np.ascontiguousarray(out)
    out[:, :, 0, :] = o0
    return out


# revision 35
# speedup vs baseline: 1.2785x; 1.1892x over previous
"""Block-local self-attention (BLOCK=128, 3-block sliding window + global token 0)
for Trainium2, sharded over 8 NeuronCores by (batch*head).

Full shapes: q/k/v (2, 16, 4096, 64) fp32, mask (2, 1, 1, 4096) fp32 (zeros).
Core c handles 4 consecutive (n*16+h) heads, as 2 "head pairs".

Design notes (keep the ScalarE exp stream continuous AND the PE row-tiled):
  - Job = (pair, window of 4 query blocks): 16 jobs/core. Scores tile is
    (128, 3072) fp32 = 6 PSUM banks, single buffer: head A pieces in banks
    0-2 ([0:1536)), head B in banks 3-5. The PE co-executes matmuls with
    disjoint row ranges out of order, and concurrent same-bank PSUM writes
    crash the device - the strict A/B bank split keeps any co-execution safe.
  - exp is split per head. The scores of job X's head A (rows 0-63) are
    emitted INTERLEAVED with job X-1's head B (rows 64-127): both are ready
    at the same time (their exp predecessors exp_A(X-1)/exp_B(X-2) have
    drained), they row-tile on the PE at full double-pumped rate, and the
    ScalarE chews ... exp_B(X-1), exp_A(X), exp_B(X) ... back to back.
  - Global token + softmax denominator are folded into the PE: per head one
    K=64 matmul (stationary rows 0-3 = pgq for the 4 query blocks, rows
    4-63 zero; rhs row r = [v0, 1] at slot r's columns) OPENS the ctx
    accumulation group writing pgq[q]*[v0,1] into each slot; the head's 12
    PV matmuls (rhs = vt with a trailing ones column) accumulate after it
    and the last one closes the group. PSUM col 64 of each slot then holds
    the softmax denominator, so normalize on DVE is just rt = 1/ctx[..,64];
    out = ctx[..,0:64] * rt (bf16, contiguous 1KB/partition stores).
  - PV/global run with a 2-job lag and normalize with a 3-job lag, so the
    ctx-tile reuse chain (PV -> norm -> next global) never blocks scores.
Query token 0 (attends the full sequence) is host-computed and patched in.
"""

import itertools
import math

import numpy as np
import ml_dtypes

N_, H, T, D = 2, 16, 4096, 64
B = 128
NB = T // B            # 32 key/query blocks
HPC = 4                # heads per core
NCORES = 8
GQ = 4                 # query blocks per job
NWIN = NB // GQ        # 8 windows per pair
SCALE = 1.0 / math.sqrt(D)
BANK = 512             # fp32 elements per PSUM bank (per partition)
VW = D + 1             # vt free width: 64 d + 1 ones
HBOFF = 3 * BANK       # head B's column offset in the scores tile (bank 3)
KG = 64                # global matmul contraction (rows 4-63 zero-padded)


def _window_pieces(w):
    """Pieces for window w: list of (j, qlo, qhi, n), q blocks absolute."""
    qb0, qb1 = GQ * w, GQ * w + GQ - 1
    out = []
    for j in range(max(0, qb0 - 1), min(NB - 1, qb1 + 1) + 1):
        qlo = max(qb0, j - 1)
        qhi = min(qb1, j + 1)
        out.append((j, qlo, qhi, (qhi - qlo + 1) * B))
    return out


def _pack_offsets(sizes):
    """Pack piece sizes contiguously from 0 s.t. no piece crosses a 512-elem
    PSUM bank boundary. Returns (offsets, total)."""
    for perm in itertools.permutations(range(len(sizes))):
        off = 0
        offs = [0] * len(sizes)
        ok = True
        for i in perm:
            sz = sizes[i]
            if off // BANK != (off + sz - 1) // BANK:
                ok = False
                break
            offs[i] = off
            off += sz
        if ok:
            return offs, off
    raise ValueError(f"cannot pack {sizes}")


_NC_CACHE = {}


def _build_nc():
    if "nc" in _NC_CACHE:
        return _NC_CACHE["nc"]

    import concourse.bacc as bacc
    import concourse.mybir as mybir
    import concourse.tile as tile

    dt = mybir.dt
    F32, BF16 = dt.float32, dt.bfloat16
    SCW = 6 * BANK  # scores tile columns (6 banks)

    nc = bacc.Bacc("TRN2", target_bir_lowering=False, debug=False)
    qt_d = nc.dram_tensor("qt", [2, 128, T], BF16, kind="ExternalInput")
    kt_d = nc.dram_tensor("kt", [2, 128, T], BF16, kind="ExternalInput")
    vt_d = nc.dram_tensor("vt", [HPC, 128, NB, VW], BF16, kind="ExternalInput")
    pgq_d = nc.dram_tensor("pgq", [HPC, KG, NWIN, B], BF16, kind="ExternalInput")
    v0one_d = nc.dram_tensor("v0one", [HPC, KG, GQ * VW], BF16, kind="ExternalInput")
    o_d = nc.dram_tensor("o", [HPC, 128, NB, D], BF16, kind="ExternalOutput")

    with tile.TileContext(nc) as tc:
        with (
            tc.tile_pool(name="singles", bufs=1) as singles,
            tc.tile_pool(name="pp", bufs=3) as pp,
            tc.tile_pool(name="rtp", bufs=2) as rtp,
            tc.tile_pool(name="outp", bufs=2) as outp,
            tc.tile_pool(name="spsum", bufs=1, space="PSUM") as spsum,
            tc.tile_pool(name="cpsum", bufs=1, space="PSUM") as cpsum,
        ):
            # Warm the ScalarE exp table first, during the DMA ramp.
            warm_in = singles.tile([1, 8], F32, tag="warm_in")
            nc.vector.memset(warm_in[:, :], 0.0)
            warm_out = singles.tile([1, 8], BF16, tag="warm_out")
            nc.scalar.activation(
                out=warm_out[:, :],
                in_=warm_in[:, :],
                func=mybir.ActivationFunctionType.Exp,
            )

            # Inputs: critical-first (pair 0's kt/qt + the globals) on the
            # idle SP queue, bulk on gpsimd SWDGE.
            vt = [None] * HPC
            SPL = 8 * B  # first chunk: kt/qt blocks 0-7 (covers 2 windows)
            kt0 = singles.tile([128, T], BF16, tag="kt0")
            qt0 = singles.tile([128, T], BF16, tag="qt0")
            kt1 = singles.tile([128, T], BF16, tag="kt1")
            qt1 = singles.tile([128, T], BF16, tag="qt1")
            kt_pair, qt_pair = [kt0, kt1], [qt0, qt1]
            pgq = [
                singles.tile([KG, NWIN, B], BF16, tag=f"pgq{h}", name=f"pgq{h}")
                for h in range(HPC)
            ]
            v0one = [
                singles.tile([KG, GQ * VW], BF16, tag=f"v0one{h}", name=f"v0one{h}")
                for h in range(HPC)
            ]

            nc.sync.dma_start(out=kt0[:, 0:SPL], in_=kt_d.ap()[0, :, 0:SPL])
            nc.sync.dma_start(out=qt0[:, 0:SPL], in_=qt_d.ap()[0, :, 0:SPL])
            for h in range(HPC):
                nc.sync.dma_start(out=pgq[h][:, :, :], in_=pgq_d.ap()[h])
                nc.sync.dma_start(out=v0one[h][:, :], in_=v0one_d.ap()[h])
            for h in (0, 1):
                vt_h = singles.tile([128, NB, VW], BF16, tag=f"vt{h}", name=f"vt{h}")
                nc.gpsimd.dma_start(out=vt_h[:, :, :], in_=vt_d.ap()[h])
                vt[h] = vt_h
            nc.gpsimd.dma_start(out=kt0[:, SPL:T], in_=kt_d.ap()[0, :, SPL:T])
            nc.gpsimd.dma_start(out=qt0[:, SPL:T], in_=qt_d.ap()[0, :, SPL:T])
            for h in (2, 3):
                vt_h = singles.tile([128, NB, VW], BF16, tag=f"vt{h}", name=f"vt{h}")
                nc.gpsimd.dma_start(out=vt_h[:, :, :], in_=vt_d.ap()[h])
                vt[h] = vt_h
            nc.gpsimd.dma_start(out=kt1[:, 0:T], in_=kt_d.ap()[1, :, 0:T])
            nc.gpsimd.dma_start(out=qt1[:, 0:T], in_=qt_d.ap()[1, :, 0:T])

            packs = []
            for w in range(NWIN):
                pieces = _window_pieces(w)
                offs, tot = _pack_offsets([p[3] for p in pieces])
                packs.append((pieces, offs, tot))

            def score_ops(job, hi):
                """(matmul-args, exp-args) for head hi of job `job`."""
                pair, w = jobs[job]
                qt, kt = qt_pair[pair], kt_pair[pair]
                dlo, hb = 64 * hi, hi * HBOFF
                pieces, offs, tot = packs[w]
                mm = []
                for (j, qlo, qhi, n), off in zip(pieces, offs):
                    mm.append(
                        (
                            (hb + off, n),
                            kt[dlo : dlo + 64, j * B : (j + 1) * B],
                            qt[dlo : dlo + 64, qlo * B : (qhi + 1) * B],
                        )
                    )
                return mm, (hb, tot)

            jobs = [(pair, w) for pair in range(2) for w in range(NWIN)]
            NJ = len(jobs)
            state_P = {}
            state_ctx = {}
            stage_cur = {}
            sc = spsum.tile([128, SCW], F32, tag="sc")
            for it in range(NJ + 3):
                # --- scores: head A of job `it` interleaved (row-tiled) with
                # head B of job `it-1`; then exp_B(it-1), exp_A(it).
                mmA = expA = mmB = expB = None
                if it < NJ:
                    mmA, expA = score_ops(it, 0)
                    state_P[it] = pp.tile([128, SCW], BF16, tag="p", name="p")
                if 0 <= it - 1 < NJ:
                    mmB, expB = score_ops(it - 1, 1)
                for a, b in itertools.zip_longest(mmA or [], mmB or []):
                    for (off, n), lhsT, rhs in (x for x in (a, b) if x):
                        nc.tensor.matmul(
                            out=sc[:, off : off + n],
                            lhsT=lhsT,
                            rhs=rhs,
                            start=True,
                            stop=True,
                        )
                if expB is not None:
                    hb, tot = expB
                    nc.scalar.activation(
                        out=state_P[it - 1][:, hb : hb + tot],
                        in_=sc[:, hb : hb + tot],
                        func=mybir.ActivationFunctionType.Exp,
                        scale=SCALE,
                    )
                if expA is not None:
                    hb, tot = expA
                    nc.scalar.activation(
                        out=state_P[it][:, hb : hb + tot],
                        in_=sc[:, hb : hb + tot],
                        func=mybir.ActivationFunctionType.Exp,
                        scale=SCALE,
                    )
                # --- PV + global for job it-2
                if 0 <= it - 2 < NJ:
                    jb = it - 2
                    pair, w = jobs[jb]
                    P = state_P.pop(jb)
                    pieces, offs, tot = packs[w]
                    off_of = {j: (off, qlo) for (j, qlo, qhi, n), off in zip(pieces, offs)}
                    ctxs = []
                    for hi in (0, 1):
                        h = 2 * pair + hi
                        hb = hi * HBOFF
                        ctx = cpsum.tile(
                            [128, GQ, VW], F32, tag=f"ctx{hi}", name=f"ctx{hi}"
                        )
                        # K=64 global matmul (rows 4+ zero) opens the bank's
                        # accumulation group: slot c += pgq[block c][q]*[v0,1]
                        nc.tensor.matmul(
                            out=ctx[:, :, :],
                            lhsT=pgq[h][:, w, :],
                            rhs=v0one[h][:, :],
                            start=True,
                            stop=False,
                        )
                        pv = []
                        for c in range(GQ):
                            cb = GQ * w + c  # absolute q block
                            js = [j for (j, qlo, qhi, n) in pieces if qlo <= cb <= qhi]
                            for j in js:
                                off, qlo = off_of[j]
                                pv.append((c, hb + off + (cb - qlo) * B, j))
                        for i, (c, col, j) in enumerate(pv):
                            nc.tensor.matmul(
                                out=ctx[:, c, :],
                                lhsT=P[:, col : col + B],
                                rhs=vt[h][:, j, :],
                                start=False,
                                stop=(i == len(pv) - 1),
                            )
                        ctxs.append(ctx)
                    state_ctx[jb] = (pair, w, ctxs)
                # --- normalize + store for job it-3
                if 0 <= it - 3 < NJ:
                    pair, w, ctxs = state_ctx.pop(it - 3)
                    wi = w % 2
                    for hi in (0, 1):
                        h = 2 * pair + hi
                        ctx = ctxs[hi]
                        if wi == 0:
                            stage_cur[h] = outp.tile(
                                [128, 2 * GQ, D], BF16, tag=f"st{h}", name=f"st{h}"
                            )
                        stage = stage_cur[h]
                        rt = rtp.tile([128, GQ], F32, tag="rt")
                        nc.vector.reciprocal(out=rt[:, :], in_=ctx[:, :, D])
                        nc.vector.tensor_mul(
                            out=stage[:, GQ * wi : GQ * wi + GQ, :],
                            in0=ctx[:, :, 0:D],
                            in1=rt[:, :].broadcast_to([128, GQ, D]),
                        )
                        if wi == 1:
                            b0 = (w - 1) * GQ
                            nc.sync.dma_start(
                                out=o_d.ap()[h, :, b0 : b0 + 2 * GQ],
                                in_=stage[:, :, :],
                            )

    nc.compile()
    _NC_CACHE["nc"] = nc
    return nc


def _host_globals(query, key, value):
    """Host-side tiny pieces: pg = exp(scale * K0 . Q) (zeroed for the first
    two query blocks), and o0 = full-sequence attention output for query 0
    (token 0 masked out, as the reference does via attention_mask[..., 0])."""
    q = np.asarray(query, np.float32)
    k = np.asarray(key, np.float32)
    v = np.asarray(value, np.float32)
    k0 = k[:, :, 0, :]  # (n, h, d)
    sg = np.einsum("nhd,nhtd->nht", k0, q) * SCALE
    pg = np.exp(sg)
    pg[:, :, : 2 * B] = 0.0

    q0 = q[:, :, 0, :]  # (n, h, d)
    s0 = np.einsum("nhd,nhtd->nht", q0, k) * SCALE
    s0[:, :, 0] = -np.inf
    s0 -= s0.max(axis=-1, keepdims=True)
    p0 = np.exp(s0)
    p0 /= p0.sum(axis=-1, keepdims=True)
    o0 = np.einsum("nht,nhtd->nhd", p0, v)
    return pg, o0


def kernel(query_layer, key_layer, value_layer, attention_mask):
    from concourse.bass_utils import run_bass_kernel_spmd

    n, h, t, d = query_layer.shape
    assert (n, h, t, d) == (N_, H, T, D)

    q = np.asarray(query_layer, np.float32)
    k = np.asarray(key_layer, np.float32)
    v = np.asarray(value_layer, np.float32)
    pg, o0 = _host_globals(q, k, v)

    bf16 = ml_dtypes.bfloat16
    qf = q.reshape(n * h, T, D)
    kf = k.reshape(n * h, T, D)
    vf = v.reshape(n * h, T, D)

    # qt/kt: per pair of heads, (128, T) bf16 = [headA dT; headB dT]
    qt_all = np.ascontiguousarray(
        qf.astype(bf16).transpose(0, 2, 1).reshape(n * h // 2, 128, T)
    )
    kt_all = np.ascontiguousarray(
        kf.astype(bf16).transpose(0, 2, 1).reshape(n * h // 2, 128, T)
    )
    # vt: (head, 128, NB, 65): [..., 0:64]=V, [..., 64]=ones
    vt_all = np.empty((n * h, 128, NB, VW), bf16)
    vt_all[:, :, :, 0:D] = vf.reshape(n * h, NB, B, D).transpose(0, 2, 1, 3)
    vt_all[:, :, :, D] = np.ones((), bf16)
    # pgq: (head, KG, NWIN, 128): stationary of the K=64 global matmul:
    # row c<GQ of window w = pg[head, query block GQ*w + c]; rows 4+ zero
    pgq_all = np.zeros((n * h, KG, NWIN, B), bf16)
    pgq_all[:, 0:GQ] = pg.reshape(n * h, NWIN, GQ, B).transpose(0, 2, 1, 3).astype(bf16)
    # v0one: (head, KG, GQ*65): row c<GQ = [v0, 1] at slot c's cols, else 0
    v0one_all = np.zeros((n * h, KG, GQ * VW), bf16)
    v0b = vf[:, 0, :].astype(bf16)
    for c in range(GQ):
        v0one_all[:, c, c * VW : c * VW + D] = v0b
        v0one_all[:, c, c * VW + D] = np.ones((), bf16)

    in_maps = []
    for c in range(NCORES):
        s = slice(HPC * c, HPC * (c + 1))
        sp = slice(HPC // 2 * c, HPC // 2 * (c + 1))
        in_maps.append(
            {
                "qt": np.ascontiguousarray(qt_all[sp]),
                "kt": np.ascontiguousarray(kt_all[sp]),
                "vt": np.ascontiguousarray(vt_all[s]),
                "pgq": np.ascontiguousarray(pgq_all[s]),
                "v0one": np.ascontiguousarray(v0one_all[s]),
            }
        )

    nc = _build_nc()
    res = run_bass_kernel_spmd(nc, in_maps, core_ids=list(range(NCORES)))
    _NC_CACHE["last_result"] = res
    out = np.concatenate([r["o"] for r in res.results], axis=0)
    out = out.astype(np.float32)
    out = out.reshape(n * h, 128, NB, D).transpose(0, 2, 1, 3).reshape(n, h, T, D)
    out = np.ascontiguousarray(out)
    out[:, :, 0, :] = o0
    return out


# revision 40
# speedup vs baseline: 1.2801x; 1.0013x over previous
"""Block-local self-attention (BLOCK=128, 3-block sliding window + global token 0)
for Trainium2, sharded over 8 NeuronCores by (batch*head).

Full shapes: q/k/v (2, 16, 4096, 64) fp32, mask (2, 1, 1, 4096) fp32 (zeros).
Core c handles 4 consecutive (n*16+h) heads, as 2 "head pairs".

Design notes (keep the ScalarE exp stream continuous AND the PE row-tiled):
  - Job = (pair, window of 4 query blocks): 16 jobs/core. Scores tile is
    (128, 3072) fp32 = 6 PSUM banks, single buffer: head A pieces in banks
    0-2 ([0:1536)), head B in banks 3-5. The PE co-executes matmuls with
    disjoint row ranges out of order, and concurrent same-bank PSUM writes
    crash the device - the strict A/B bank split keeps any co-execution safe.
  - exp is split per head. The scores of job X's head A (rows 0-63) are
    emitted INTERLEAVED with job X-1's head B (rows 64-127): both are ready
    at the same time (their exp predecessors exp_A(X-1)/exp_B(X-2) have
    drained), they row-tile on the PE at full double-pumped rate, and the
    ScalarE chews ... exp_B(X-1), exp_A(X), exp_B(X) ... back to back.
  - Global token + softmax denominator are folded into the PE: per head one
    K=64 matmul (stationary rows 0-3 = pgq for the 4 query blocks, rows
    4-63 zero; rhs row r = [v0, 1] at slot r's columns) OPENS the ctx
    accumulation group writing pgq[q]*[v0,1] into each slot; the head's 12
    PV matmuls (rhs = vt with a trailing ones column) accumulate after it
    and the last one closes the group. PSUM col 64 of each slot then holds
    the softmax denominator, so normalize on DVE is just rt = 1/ctx[..,64];
    out = ctx[..,0:64] * rt (bf16, contiguous 1KB/partition stores).
  - PV/global run with a 2-job lag and normalize with a 3-job lag, so the
    ctx-tile reuse chain (PV -> norm -> next global) never blocks scores.
Query token 0 (attends the full sequence) is host-computed and patched in.
"""

import itertools
import math

import numpy as np
import ml_dtypes

N_, H, T, D = 2, 16, 4096, 64
B = 128
NB = T // B            # 32 key/query blocks
HPC = 4                # heads per core
NCORES = 8
GQ = 4                 # query blocks per job
NWIN = NB // GQ        # 8 windows per pair
SCALE = 1.0 / math.sqrt(D)
BANK = 512             # fp32 elements per PSUM bank (per partition)
VW = D + 1             # vt free width: 64 d + 1 ones
HBOFF = 3 * BANK       # head B's column offset in the scores tile (bank 3)
KG = 64                # global matmul contraction (rows 4-63 zero-padded)


def _window_pieces(w):
    """Pieces for window w: list of (j, qlo, qhi, n), q blocks absolute."""
    qb0, qb1 = GQ * w, GQ * w + GQ - 1
    out = []
    for j in range(max(0, qb0 - 1), min(NB - 1, qb1 + 1) + 1):
        qlo = max(qb0, j - 1)
        qhi = min(qb1, j + 1)
        out.append((j, qlo, qhi, (qhi - qlo + 1) * B))
    return out


def _pack_offsets(sizes):
    """Pack piece sizes contiguously from 0 s.t. no piece crosses a 512-elem
    PSUM bank boundary. Returns (offsets, total)."""
    for perm in itertools.permutations(range(len(sizes))):
        off = 0
        offs = [0] * len(sizes)
        ok = True
        for i in perm:
            sz = sizes[i]
            if off // BANK != (off + sz - 1) // BANK:
                ok = False
                break
            offs[i] = off
            off += sz
        if ok:
            return offs, off
    raise ValueError(f"cannot pack {sizes}")


_NC_CACHE = {}


def _build_nc():
    if "nc" in _NC_CACHE:
        return _NC_CACHE["nc"]

    import concourse.bacc as bacc
    import concourse.mybir as mybir
    import concourse.tile as tile

    dt = mybir.dt
    F32, BF16 = dt.float32, dt.bfloat16
    SCW = 6 * BANK  # scores tile columns (6 banks)

    nc = bacc.Bacc("TRN2", target_bir_lowering=False, debug=False)
    qt_d = nc.dram_tensor("qt", [2, 128, T], BF16, kind="ExternalInput")
    kt_d = nc.dram_tensor("kt", [2, 128, T], BF16, kind="ExternalInput")
    vt_d = nc.dram_tensor("vt", [HPC, 128, NB, VW], BF16, kind="ExternalInput")
    pgq_d = nc.dram_tensor("pgq", [HPC, GQ, NWIN, B], BF16, kind="ExternalInput")
    v0one_d = nc.dram_tensor("v0one", [HPC, GQ, GQ * VW], BF16, kind="ExternalInput")
    o_d = nc.dram_tensor("o", [HPC, 128, NB, D], BF16, kind="ExternalOutput")

    with tile.TileContext(nc) as tc:
        with (
            tc.tile_pool(name="singles", bufs=1) as singles,
            tc.tile_pool(name="pp", bufs=3) as pp,
            tc.tile_pool(name="rtp", bufs=2) as rtp,
            tc.tile_pool(name="outp", bufs=2) as outp,
            tc.tile_pool(name="spsum", bufs=1, space="PSUM") as spsum,
            tc.tile_pool(name="cpsum", bufs=1, space="PSUM") as cpsum,
        ):
            # Warm the ScalarE exp table first, during the DMA ramp.
            warm_in = singles.tile([1, 8], F32, tag="warm_in")
            nc.vector.memset(warm_in[:, :], 0.0)
            warm_out = singles.tile([1, 8], BF16, tag="warm_out")
            nc.scalar.activation(
                out=warm_out[:, :],
                in_=warm_in[:, :],
                func=mybir.ActivationFunctionType.Exp,
            )

            # Inputs: critical-first (pair 0's kt/qt + the globals) on the
            # idle SP queue, bulk on gpsimd SWDGE.
            vt = [None] * HPC
            SPL = 8 * B  # first chunk: kt/qt blocks 0-7 (covers 2 windows)
            kt0 = singles.tile([128, T], BF16, tag="kt0")
            qt0 = singles.tile([128, T], BF16, tag="qt0")
            kt1 = singles.tile([128, T], BF16, tag="kt1")
            qt1 = singles.tile([128, T], BF16, tag="qt1")
            kt_pair, qt_pair = [kt0, kt1], [qt0, qt1]
            pgq = [
                singles.tile([KG, NWIN, B], BF16, tag=f"pgq{h}", name=f"pgq{h}")
                for h in range(HPC)
            ]
            v0one = [
                singles.tile([KG, GQ * VW], BF16, tag=f"v0one{h}", name=f"v0one{h}")
                for h in range(HPC)
            ]

            nc.sync.dma_start(out=kt0[:, 0:SPL], in_=kt_d.ap()[0, :, 0:SPL])
            nc.sync.dma_start(out=qt0[:, 0:SPL], in_=qt_d.ap()[0, :, 0:SPL])
            for h in range(HPC):
                # zero the whole stationary, then land rows 0:GQ (real data)
                nc.vector.memset(pgq[h][:, :, :], 0.0)
                nc.vector.memset(v0one[h][:, :], 0.0)
                nc.sync.dma_start(out=pgq[h][0:GQ, :, :], in_=pgq_d.ap()[h])
                nc.sync.dma_start(out=v0one[h][0:GQ, :], in_=v0one_d.ap()[h])
            for h in (0, 1):
                vt_h = singles.tile([128, NB, VW], BF16, tag=f"vt{h}", name=f"vt{h}")
                nc.gpsimd.dma_start(out=vt_h[:, :, :], in_=vt_d.ap()[h])
                vt[h] = vt_h
            nc.gpsimd.dma_start(out=kt0[:, SPL:T], in_=kt_d.ap()[0, :, SPL:T])
            nc.gpsimd.dma_start(out=qt0[:, SPL:T], in_=qt_d.ap()[0, :, SPL:T])
            for h in (2, 3):
                vt_h = singles.tile([128, NB, VW], BF16, tag=f"vt{h}", name=f"vt{h}")
                nc.gpsimd.dma_start(out=vt_h[:, :, :], in_=vt_d.ap()[h])
                vt[h] = vt_h
            nc.gpsimd.dma_start(out=kt1[:, 0:T], in_=kt_d.ap()[1, :, 0:T])
            nc.gpsimd.dma_start(out=qt1[:, 0:T], in_=qt_d.ap()[1, :, 0:T])

            packs = []
            for w in range(NWIN):
                pieces = _window_pieces(w)
                offs, tot = _pack_offsets([p[3] for p in pieces])
                packs.append((pieces, offs, tot))

            def score_ops(job, hi):
                """(matmul-args, exp-args) for head hi of job `job`."""
                pair, w = jobs[job]
                qt, kt = qt_pair[pair], kt_pair[pair]
                dlo, hb = 64 * hi, hi * HBOFF
                pieces, offs, tot = packs[w]
                mm = []
                for (j, qlo, qhi, n), off in zip(pieces, offs):
                    mm.append(
                        (
                            (hb + off, n),
                            kt[dlo : dlo + 64, j * B : (j + 1) * B],
                            qt[dlo : dlo + 64, qlo * B : (qhi + 1) * B],
                        )
                    )
                return mm, (hb, tot)

            jobs = [(pair, w) for pair in range(2) for w in range(NWIN)]
            NJ = len(jobs)
            state_P = {}
            state_ctx = {}
            stage_cur = {}
            sc = spsum.tile([128, SCW], F32, tag="sc")
            for it in range(NJ + 3):
                # --- scores: head A of job `it` interleaved (row-tiled) with
                # head B of job `it-1`; then exp_B(it-1), exp_A(it).
                mmA = expA = mmB = expB = None
                if it < NJ:
                    mmA, expA = score_ops(it, 0)
                    state_P[it] = pp.tile([128, SCW], BF16, tag="p", name="p")
                if 0 <= it - 1 < NJ:
                    mmB, expB = score_ops(it - 1, 1)
                # B(it-1) first: it unlocks at exp_B(it-2), a full exp
                # earlier than A(it) does (exp_A(it-1)), so emitting it as a
                # whole block lets it stream during exp_A(it-1) instead of
                # stalling behind A pieces (PE issue is in-order).
                for (off, n), lhsT, rhs in (mmB or []) + (mmA or []):
                    nc.tensor.matmul(
                        out=sc[:, off : off + n],
                        lhsT=lhsT,
                        rhs=rhs,
                        start=True,
                        stop=True,
                    )
                if expB is not None:
                    hb, tot = expB
                    nc.scalar.activation(
                        out=state_P[it - 1][:, hb : hb + tot],
                        in_=sc[:, hb : hb + tot],
                        func=mybir.ActivationFunctionType.Exp,
                        scale=SCALE,
                    )
                if expA is not None:
                    hb, tot = expA
                    nc.scalar.activation(
                        out=state_P[it][:, hb : hb + tot],
                        in_=sc[:, hb : hb + tot],
                        func=mybir.ActivationFunctionType.Exp,
                        scale=SCALE,
                    )
                # --- PV + global for job it-2
                if 0 <= it - 2 < NJ:
                    jb = it - 2
                    pair, w = jobs[jb]
                    P = state_P.pop(jb)
                    pieces, offs, tot = packs[w]
                    off_of = {j: (off, qlo) for (j, qlo, qhi, n), off in zip(pieces, offs)}
                    ctxs = []
                    for hi in (0, 1):
                        h = 2 * pair + hi
                        hb = hi * HBOFF
                        ctx = cpsum.tile(
                            [128, GQ, VW], F32, tag=f"ctx{hi}", name=f"ctx{hi}"
                        )
                        # K=64 global matmul (rows 4+ zero) opens the bank's
                        # accumulation group: slot c += pgq[block c][q]*[v0,1]
                        nc.tensor.matmul(
                            out=ctx[:, :, :],
                            lhsT=pgq[h][:, w, :],
                            rhs=v0one[h][:, :],
                            start=True,
                            stop=False,
                        )
                        pv = []
                        for c in range(GQ):
                            cb = GQ * w + c  # absolute q block
                            js = [j for (j, qlo, qhi, n) in pieces if qlo <= cb <= qhi]
                            for j in js:
                                off, qlo = off_of[j]
                                pv.append((c, hb + off + (cb - qlo) * B, j))
                        for i, (c, col, j) in enumerate(pv):
                            nc.tensor.matmul(
                                out=ctx[:, c, :],
                                lhsT=P[:, col : col + B],
                                rhs=vt[h][:, j, :],
                                start=False,
                                stop=(i == len(pv) - 1),
                            )
                        ctxs.append(ctx)
                    state_ctx[jb] = (pair, w, ctxs)
                # --- normalize + store for job it-3
                if 0 <= it - 3 < NJ:
                    pair, w, ctxs = state_ctx.pop(it - 3)
                    wi = w % 2
                    for hi in (0, 1):
                        h = 2 * pair + hi
                        ctx = ctxs[hi]
                        if wi == 0:
                            stage_cur[h] = outp.tile(
                                [128, 2 * GQ, D], BF16, tag=f"st{h}", name=f"st{h}"
                            )
                        stage = stage_cur[h]
                        rt = rtp.tile([128, GQ], F32, tag="rt")
                        nc.vector.reciprocal(out=rt[:, :], in_=ctx[:, :, D])
                        nc.vector.tensor_mul(
                            out=stage[:, GQ * wi : GQ * wi + GQ, :],
                            in0=ctx[:, :, 0:D],
                            in1=rt[:, :].broadcast_to([128, GQ, D]),
                        )
                        if wi == 1:
                            b0 = (w - 1) * GQ
                            nc.sync.dma_start(
                                out=o_d.ap()[h, :, b0 : b0 + 2 * GQ],
                                in_=stage[:, :, :],
                            )

    nc.compile()
    _NC_CACHE["nc"] = nc
    return nc


def _host_globals(query, key, value):
    """Host-side tiny pieces: pg = exp(scale * K0 . Q) (zeroed for the first
    two query blocks), and o0 = full-sequence attention output for query 0
    (token 0 masked out, as the reference does via attention_mask[..., 0])."""
    q = np.asarray(query, np.float32)
    k = np.asarray(key, np.float32)
    v = np.asarray(value, np.float32)
    k0 = k[:, :, 0, :]  # (n, h, d)
    sg = np.einsum("nhd,nhtd->nht", k0, q) * SCALE
    pg = np.exp(sg)
    pg[:, :, : 2 * B] = 0.0

    q0 = q[:, :, 0, :]  # (n, h, d)
    s0 = np.einsum("nhd,nhtd->nht", q0, k) * SCALE
    s0[:, :, 0] = -np.inf
    s0 -= s0.max(axis=-1, keepdims=True)
    p0 = np.exp(s0)
    p0 /= p0.sum(axis=-1, keepdims=True)
    o0 = np.einsum("nht,nhtd->nhd", p0, v)
    return pg, o0


def kernel(query_layer, key_layer, value_layer, attention_mask):
    from concourse.bass_utils import run_bass_kernel_spmd

    n, h, t, d = query_layer.shape
    assert (n, h, t, d) == (N_, H, T, D)

    q = np.asarray(query_layer, np.float32)
    k = np.asarray(key_layer, np.float32)
    v = np.asarray(value_layer, np.float32)
    pg, o0 = _host_globals(q, k, v)

    bf16 = ml_dtypes.bfloat16
    qf = q.reshape(n * h, T, D)
    kf = k.reshape(n * h, T, D)
    vf = v.reshape(n * h, T, D)

    # qt/kt: per pair of heads, (128, T) bf16 = [headA dT; headB dT]
    qt_all = np.ascontiguousarray(
        qf.astype(bf16).transpose(0, 2, 1).reshape(n * h // 2, 128, T)
    )
    kt_all = np.ascontiguousarray(
        kf.astype(bf16).transpose(0, 2, 1).reshape(n * h // 2, 128, T)
    )
    # vt: (head, 128, NB, 65): [..., 0:64]=V, [..., 64]=ones
    vt_all = np.empty((n * h, 128, NB, VW), bf16)
    vt_all[:, :, :, 0:D] = vf.reshape(n * h, NB, B, D).transpose(0, 2, 1, 3)
    vt_all[:, :, :, D] = np.ones((), bf16)
    # pgq: (head, GQ, NWIN, 128): real rows of the K=64 global matmul
    # stationary: row c of window w = pg[head, query block GQ*w + c]
    # (rows GQ:KG are zeroed on device)
    pgq_all = np.ascontiguousarray(
        pg.reshape(n * h, NWIN, GQ, B).transpose(0, 2, 1, 3).astype(bf16)
    )
    # v0one: (head, GQ, GQ*65): row c = [v0, 1] at slot c's cols, else 0
    v0one_all = np.zeros((n * h, GQ, GQ * VW), bf16)
    v0b = vf[:, 0, :].astype(bf16)
    for c in range(GQ):
        v0one_all[:, c, c * VW : c * VW + D] = v0b
        v0one_all[:, c, c * VW + D] = np.ones((), bf16)

    in_maps = []
    for c in range(NCORES):
        s = slice(HPC * c, HPC * (c + 1))
        sp = slice(HPC // 2 * c, HPC // 2 * (c + 1))
        in_maps.append(
            {
                "qt": np.ascontiguousarray(qt_all[sp]),
                "kt": np.ascontiguousarray(kt_all[sp]),
                "vt": np.ascontiguousarray(vt_all[s]),
                "pgq": np.ascontiguousarray(pgq_all[s]),
                "v0one": np.ascontiguousarray(v0one_all[s]),
            }
        )

    nc = _build_nc()
    res = run_bass_kernel_spmd(nc, in_maps, core_ids=list(range(NCORES)))
    _NC_CACHE["last_result"] = res
    out = np.concatenate([r["o"] for r in res.results], axis=0)
    out = out.astype(np.float32)
    out = out.reshape(n * h, 128, NB, D).transpose(0, 2, 1, 3).reshape(n, h, T, D)
    out = np.ascontiguousarray(out)
    out[:, :, 0, :] = o0
    return out


# revision 46
# speedup vs baseline: 1.4791x; 1.1555x over previous
"""Block-local self-attention (BLOCK=128, 3-block sliding window + global token 0)
for Trainium2, sharded over 8 NeuronCores by (batch*head).

Full shapes: q/k/v (2, 16, 4096, 64) fp32, mask (2, 1, 1, 4096) fp32 (zeros).
Core c handles 4 consecutive (n*16+h) heads, as 2 "head pairs".

Host prepares compute-ready, DMA-friendly layouts (big contiguous descriptors):
  - qt/kt: (pair, 128, T) bf16: rows 0-63 = head A's d, 64-127 = head B's d.
  - vt: (head, 128, NB, 65) bf16: partition = token%128, free = (block, d);
    col 64 = ones (softmax denominator trick).
  - pgq: (head, 128, NB) bf16: exp(scale*K0.Q) in q-partition layout,
    zeroed for query blocks 0,1 (global-token probability, host-computed).
  - v0g: (head, 128, 4, 64) bf16: V[token 0] replicated across partitions
    and the 4 query blocks of a window (for the DVE outer-product add).
  - o: (head, 128, NB, D) fp32 staging layout; host un-permutes after.

Device kernel per (pair, window of 512 queries), software-pipelined:
  - scores for BOTH heads in one (128, 3072) PSUM tile, S^T (key-partition)
    layout: per key block j, two row-tiled matmuls (head A on array rows
    0-63, head B on rows 64-127) run concurrently in the PE array.
  - one exp on ScalarE (scale folded) -> P^T bf16 (128, 3072).
  - PV in q-partition layout: for each (query block c, key block j),
    matmul(lhsT=P_j[:, c cols], rhs=vt_j) accumulates ctx (128, 4, 65)
    PSUM; col 64 = denominator via the ones column. N=65 per matmul, so
    PV streams 780 cols/head/window instead of 1536.
  - normalize on DVE directly from PSUM: denom += pgq, reciprocal,
    ctx += pgq (x) V0 (global-token term), multiply -> fp32 out staging.
Query token 0 (attends the full sequence) is host-computed and patched in.
"""

import math

import numpy as np
import ml_dtypes

N_, H, T, D = 2, 16, 4096, 64
B = 128
NB = T // B            # 32 key/query blocks
HPC = 4                # heads per core
NCORES = 8
WQ = 512               # queries per window
NWIN = T // WQ         # 8 windows per head
SCALE = 1.0 / math.sqrt(D)
BANK = 512             # fp32 elements per PSUM bank (per partition)
VW = D + 1             # vt free width: 64 d + 1 ones


def _window_pieces(w):
    """Pieces for window w: (j, qb_lo, qb_hi, N) with q blocks absolute."""
    qb0, qb1 = 4 * w, 4 * w + 3
    out = []
    for j in range(max(0, qb0 - 1), min(NB - 1, qb1 + 1) + 1):
        qlo = max(qb0, j - 1)
        qhi = min(qb1, j + 1)
        out.append((j, qlo, qhi, (qhi - qlo + 1) * B))
    return out


def _pack_offsets(sizes):
    """Pack piece sizes contiguously from 0 s.t. no piece crosses a 512-elem
    PSUM bank boundary. Returns list of offsets (same order as sizes)."""
    import itertools

    n = len(sizes)
    for perm in itertools.permutations(range(n)):
        off = 0
        offs = [0] * n
        ok = True
        for i in perm:
            sz = sizes[i]
            if off // BANK != (off + sz - 1) // BANK:
                ok = False
                break
            offs[i] = off
            off += sz
        if ok:
            return offs
    raise ValueError(f"cannot pack {sizes}")


_NC_CACHE = {}


def _build_nc():
    if "nc" in _NC_CACHE:
        return _NC_CACHE["nc"]

    import concourse.bacc as bacc
    import concourse.mybir as mybir
    import concourse.tile as tile

    dt = mybir.dt
    F32, BF16 = dt.float32, dt.bfloat16
    HB = 3 * BANK  # per-head columns in the scores tile

    nc = bacc.Bacc("TRN2", target_bir_lowering=False, debug=False)
    qt_d = nc.dram_tensor("qt", [2, 128, T], BF16, kind="ExternalInput")
    kt_d = nc.dram_tensor("kt", [2, 128, T], BF16, kind="ExternalInput")
    vt_d = nc.dram_tensor("vt", [HPC, 128, NB, VW], BF16, kind="ExternalInput")
    pgq_d = nc.dram_tensor("pgq", [HPC, 128, NB], BF16, kind="ExternalInput")
    v0g_d = nc.dram_tensor("v0g", [HPC, 128, 4, D], BF16, kind="ExternalInput")
    o_d = nc.dram_tensor("o", [HPC, 128, NB, D], BF16, kind="ExternalOutput")

    with tile.TileContext(nc) as tc:
        with (
            tc.tile_pool(name="singles", bufs=1) as singles,
            tc.tile_pool(name="pp", bufs=2) as pp,
            tc.tile_pool(name="gp", bufs=2) as gp,
            tc.tile_pool(name="up", bufs=2) as up,
            tc.tile_pool(name="rtp", bufs=2) as rtp,
            tc.tile_pool(name="outp", bufs=1) as outp,
            tc.tile_pool(name="spsum", bufs=1, space="PSUM") as spsum,
            tc.tile_pool(name="cpsum", bufs=2, space="PSUM") as cpsum,
        ):
            # Input loads: plain SWDGE (gpsimd) big contiguous transfers,
            # ordered so pair-0 compute starts ASAP. qt/kt split in two
            # chunks so the first window's blocks arrive early.
            qt_pair, kt_pair = [None] * 2, [None] * 2
            vt, pgq, v0g = [None] * HPC, [None] * HPC, [None] * HPC
            SPL = 6 * B  # first chunk: blocks 0-5 (covers window 0)
            for pair in range(2):
                hA, hB = 2 * pair, 2 * pair + 1
                kt = singles.tile([128, T], BF16, tag=f"kt{pair}")
                qt = singles.tile([128, T], BF16, tag=f"qt{pair}")
                if pair == 0:
                    nc.sync.dma_start(out=kt[:, 0:SPL], in_=kt_d.ap()[0, :, 0:SPL])
                    nc.scalar.dma_start(out=qt[:, 0:SPL], in_=qt_d.ap()[0, :, 0:SPL])
                else:
                    nc.gpsimd.dma_start(out=kt[:, 0:SPL], in_=kt_d.ap()[1, :, 0:SPL])
                    nc.gpsimd.dma_start(out=qt[:, 0:SPL], in_=qt_d.ap()[1, :, 0:SPL])
                nc.gpsimd.dma_start(out=kt[:, SPL:T], in_=kt_d.ap()[pair, :, SPL:T])
                nc.gpsimd.dma_start(out=qt[:, SPL:T], in_=qt_d.ap()[pair, :, SPL:T])
                kt_pair[pair], qt_pair[pair] = kt, qt
                for h in (hA, hB):
                    vt_h = singles.tile([128, NB, VW], BF16, tag=f"vt{h}")
                    nc.gpsimd.dma_start(out=vt_h[:, :, :], in_=vt_d.ap()[h])
                    vt[h] = vt_h
                    pgq_h = singles.tile([128, NB], BF16, tag=f"pgq{h}")
                    nc.gpsimd.dma_start(out=pgq_h[:, :], in_=pgq_d.ap()[h])
                    pgq[h] = pgq_h
                    v0g_h = singles.tile([128, 4, D], BF16, tag=f"v0g{h}")
                    nc.gpsimd.dma_start(out=v0g_h[:, :, :], in_=v0g_d.ap()[h])
                    v0g[h] = v0g_h

            # Warm the ScalarE exp table during the DMA ramp.
            warm_in = singles.tile([1, 8], F32, tag="warm_in")
            nc.vector.memset(warm_in[:, :], 0.0)
            warm_out = singles.tile([1, 8], BF16, tag="warm_out")
            nc.scalar.activation(
                out=warm_out[:, :],
                in_=warm_in[:, :],
                func=mybir.ActivationFunctionType.Exp,
            )

            # Output staging in 8-block tiles matching store granularity,
            # so a store in flight never blocks the next windows' normalize
            # writes (Tile tracks deps per tile).
            outstage = []
            for h in range(HPC):
                tiles_h = []
                for s in range(4):
                    out_hs = outp.tile([128, 8, D], BF16, tag=f"out{h}_{s}")
                    tiles_h.append(out_hs)
                outstage.append(tiles_h)

            # Compute, software-pipelined across a flat (pair, window) job
            # list: at step `it` we emit scores+exp for job it, PV for job
            # it-1, normalize+store for it-2.
            jobs = [(pair, w) for pair in range(2) for w in range(NWIN)]
            state = {}
            for it in range(len(jobs) + 2):
                if it < len(jobs):
                    pair, w = jobs[it]
                    qt, kt = qt_pair[pair], kt_pair[pair]
                    pieces = _window_pieces(w)
                    offs = _pack_offsets([p[3] for p in pieces])
                    tot = sum(p[3] for p in pieces)
                    sc = spsum.tile([128, 2 * HB], F32, tag="sc")
                    # Row-tiled pairs: head A on array rows 0-63 writes
                    # cols [0:HB), head B on rows 64-127 writes [HB:2HB).
                    # Adjacent emission lets the PE run them concurrently.
                    for (j, qlo, qhi, n), off in zip(pieces, offs):
                        for hi, dlo in ((0, 0), (1, 64)):
                            nc.tensor.matmul(
                                out=sc[:, hi * HB + off : hi * HB + off + n],
                                lhsT=kt[dlo : dlo + 64, j * B : (j + 1) * B],
                                rhs=qt[dlo : dlo + 64, qlo * B : (qhi + 1) * B],
                                start=True,
                                stop=True,
                            )
                    P = pp.tile([128, 2 * HB], BF16, tag="p")
                    nc.scalar.activation(
                        out=P[:, 0 : HB + tot],
                        in_=sc[:, 0 : HB + tot],
                        func=mybir.ActivationFunctionType.Exp,
                        scale=SCALE,
                    )
                    state[it] = (pair, w, pieces, offs, P)
                if 0 <= it - 1 < len(jobs):
                    pair, w, pieces, offs, P = state[it - 1]
                    ctxs = []
                    for hi, h in ((0, 2 * pair), (1, 2 * pair + 1)):
                        ctx = cpsum.tile([128, 4, VW], F32, tag="ctx")
                        for c in range(4):
                            cb = 4 * w + c  # absolute q block
                            js = [j for (j, qlo, qhi, n) in pieces if qlo <= cb <= qhi]
                            for ji, j in enumerate(js):
                                (jj, qlo, qhi, n), off = next(
                                    (pc, of) for pc, of in zip(pieces, offs) if pc[0] == j
                                )
                                col = hi * HB + off + (cb - qlo) * B
                                nc.tensor.matmul(
                                    out=ctx[:, c, :],
                                    lhsT=P[:, col : col + B],
                                    rhs=vt[h][:, j, :],
                                    start=(ji == 0),
                                    stop=(ji == len(js) - 1),
                                )
                        ctxs.append(ctx)
                    state[it - 1] = (pair, w, ctxs)
                if 0 <= it - 2 < len(jobs):
                    pair, w, ctxs = state.pop(it - 2)
                    for hi, h in ((0, 2 * pair), (1, 2 * pair + 1)):
                        ctx = ctxs[hi]
                        pslice = pgq[h][:, 4 * w : 4 * w + 4]
                        # g first (SBUF-only), then the two ctx readers
                        # back-to-back so the PSUM bank frees ASAP for the
                        # next job's PV.
                        g = gp.tile([128, 4, D], BF16, tag="g")
                        nc.vector.tensor_mul(
                            out=g[:, :, :],
                            in0=v0g[h][:, :, :],
                            in1=pslice.broadcast_to([128, 4, D]),
                        )
                        dn = rtp.tile([128, 4], F32, tag="dn")
                        nc.vector.tensor_tensor(
                            out=dn[:, :],
                            in0=ctx[:, :, D],
                            in1=pslice,
                            op=mybir.AluOpType.add,
                        )
                        u = up.tile([128, 4, D], F32, tag="u")
                        nc.vector.tensor_tensor(
                            out=u[:, :, :],
                            in0=ctx[:, :, 0:D],
                            in1=g[:, :, :],
                            op=mybir.AluOpType.add,
                        )
                        rt = rtp.tile([128, 4], F32, tag="rt")
                        nc.vector.reciprocal(out=rt[:, :], in_=dn[:, :])
                        ostage = outstage[h][w // 2]
                        nc.vector.tensor_mul(
                            out=ostage[:, (w % 2) * 4 : (w % 2) * 4 + 4, :],
                            in0=u[:, :, :],
                            in1=rt[:, :].broadcast_to([128, 4, D]),
                        )
                        if w % 2 == 1:
                            b0 = (w - 1) * 4
                            nc.sync.dma_start(
                                out=o_d.ap()[h, :, b0 : b0 + 8],
                                in_=ostage[:, :, :],
                            )

    nc.compile()
    _NC_CACHE["nc"] = nc
    return nc


def _host_globals(query, key, value):
    """Host-side tiny pieces: pg = exp(scale * K0 . Q) (zeroed for the first
    two query blocks), and o0 = full-sequence attention output for query 0
    (token 0 masked out, as the reference does via attention_mask[..., 0])."""
    q = np.asarray(query, np.float32)
    k = np.asarray(key, np.float32)
    v = np.asarray(value, np.float32)
    k0 = k[:, :, 0, :]  # (n, h, d)
    sg = np.einsum("nhd,nhtd->nht", k0, q) * SCALE
    pg = np.exp(sg)
    pg[:, :, : 2 * B] = 0.0

    q0 = q[:, :, 0, :]  # (n, h, d)
    s0 = np.einsum("nhd,nhtd->nht", q0, k) * SCALE
    s0[:, :, 0] = -np.inf
    s0 -= s0.max(axis=-1, keepdims=True)
    p0 = np.exp(s0)
    p0 /= p0.sum(axis=-1, keepdims=True)
    o0 = np.einsum("nht,nhtd->nhd", p0, v)
    return pg, o0


def kernel(query_layer, key_layer, value_layer, attention_mask):
    from concourse.bass_utils import run_bass_kernel_spmd

    n, h, t, d = query_layer.shape
    assert (n, h, t, d) == (N_, H, T, D)

    q = np.asarray(query_layer, np.float32)
    k = np.asarray(key_layer, np.float32)
    v = np.asarray(value_layer, np.float32)
    pg, o0 = _host_globals(q, k, v)

    bf16 = ml_dtypes.bfloat16
    qf = q.reshape(n * h, T, D)
    kf = k.reshape(n * h, T, D)
    vf = v.reshape(n * h, T, D)

    # qt/kt: per pair of heads, (128, T) bf16 = [headA dT; headB dT]
    qt_all = np.ascontiguousarray(
        qf.astype(bf16).transpose(0, 2, 1).reshape(n * h // 2, 128, T)
    )
    kt_all = np.ascontiguousarray(
        kf.astype(bf16).transpose(0, 2, 1).reshape(n * h // 2, 128, T)
    )
    # vt: (head, 128, NB, 65): [..., 0:64]=V, [..., 64]=ones
    vt_all = np.empty((n * h, 128, NB, VW), bf16)
    vt_all[:, :, :, 0:D] = vf.reshape(n * h, NB, B, D).transpose(0, 2, 1, 3)
    vt_all[:, :, :, D] = np.ones((), bf16)
    # pgq: (head, 128, NB) = pg in q-partition layout
    pgq_all = np.ascontiguousarray(
        pg.reshape(n * h, NB, B).transpose(0, 2, 1).astype(bf16)
    )
    # v0g: (head, 128, 4, 64) = V[0] replicated
    v0g_all = np.ascontiguousarray(
        np.broadcast_to(
            vf[:, 0, :].astype(bf16)[:, None, None, :], (n * h, 128, 4, D)
        )
    )

    in_maps = []
    for c in range(NCORES):
        s = slice(HPC * c, HPC * (c + 1))
        sp = slice(HPC // 2 * c, HPC // 2 * (c + 1))
        in_maps.append(
            {
                "qt": np.ascontiguousarray(qt_all[sp]),
                "kt": np.ascontiguousarray(kt_all[sp]),
                "vt": np.ascontiguousarray(vt_all[s]),
                "pgq": np.ascontiguousarray(pgq_all[s]),
                "v0g": np.ascontiguousarray(v0g_all[s]),
            }
        )

    nc = _build_nc()
    res = run_bass_kernel_spmd(nc, in_maps, core_ids=list(range(NCORES)))
    _NC_CACHE["last_result"] = res
    out = np.concatenate([r["o"] for r in res.results], axis=0)
    out = out.astype(np.float32)
    out = out.reshape(n * h, 128, NB, D).transpose(0, 2, 1, 3).reshape(n, h, T, D)
    out = np.ascontiguousarray(out)
    out[:, :, 0, :] = o0
    return out

